# revision 32
# baseline (speedup 1.0000x reference)
"""GroupMixAttention Trainium2 kernel (8-core SPMD, batch-parallel), v2.

Problem: x[16,256,32,32]; per group g (4 groups of 64 ch):
  Q/K/V = wq/wk/wv[g] @ xg   (xg = [64, 1024])
  scores = (Q^T K)/8 ; attn = softmax(scores, -1) ; out = V @ attn^T
then y = wo @ concat(out).

Sharding: data-parallel over batch, 2 batches per core, no collectives.

v2 design (from baseline trace analysis: PE busy 170us of 211us span,
everything fp32 at half stream rate; ACT exp 79us):
  - whole pipeline in fp16 (host casts inputs): matmuls stream at full
    rate (1 col/cycle @2.4GHz vs 1/2 for fp32).
  - K projection eliminated algebraically: scores^T = K^T Q
    = x^T (wk^T wq) x, so host precomputes A^T = wq^T wk per group and
    the kernel computes P = A x (one projection) then scores chunks
    directly from raw x chunks as lhsT.
  - both groups of a pair are packed in PE row-bands (tile_position
    (0,0)/(64,0)) writing the two 512-col halves of one [128,1024]
    PSUM tile -> ONE exp per m-chunk step (halves ACT instr overhead).
  - attention inner loop is ACT(exp)-bound (~1.05us/step vs ~0.85us of
    PE work); the next unit's P-projection and V^T matmuls are
    software-injected into the loop's PE slack, and out_proj of the
    previous batch is placed at nh boundaries where the exp pipeline
    drains anyway.
  - softmax denominators via ones-column in the PV lhsT (row 64 of
    psO); normalization = DVE reciprocal_approx_fast on the den row +
    gpsimd partition_broadcast + gpsimd multiply (gpsimd is otherwise
    idle; PE no longer does broadcast matmuls).
  - V^T chunks accumulate into [128,8,64] PSUM tiles; one strided DVE
    cast per group fills the [128,8,65] fp16 lhsT (ones col memset).
"""

import os
import sys

import numpy as np

for _p in ("/opt/trn_rl_repo", "/root/.axon_site/_ro/trn_rl_repo"):
    if os.path.isdir(_p) and _p not in sys.path:
        sys.path.insert(0, _p)

import concourse.bass as bass
import concourse.mybir as mybir
import concourse.tile as tile
from concourse import bacc
from concourse.bass_utils import run_bass_kernel_spmd

F32 = mybir.dt.float32
F16 = mybir.dt.float16
EXP = mybir.ActivationFunctionType.Exp
N_CORES = 8
B_PER_CORE = 2  # 16 batches / 8 cores
N_UNITS = 4     # (batch, group-pair) units per core
NT = 1024       # H*W
GD = 64         # group dim
ts = bass.ts


def _build_program():
    nc = bacc.Bacc("TRN2", target_bir_lowering=False, debug=False,
                   num_devices=N_CORES)
    xs = nc.dram_tensor("xs", [N_UNITS, 128, NT], F16,
                        kind="ExternalInput").ap()
    wAT = nc.dram_tensor("wAT", [2, 128, GD], F16, kind="ExternalInput").ap()
    wvT = nc.dram_tensor("wvT", [2, 128, GD], F16, kind="ExternalInput").ap()
    woT = nc.dram_tensor("woT", [2, 128, 256], F16, kind="ExternalInput").ap()
    y = nc.dram_tensor("y", [B_PER_CORE, 256, NT], F16,
                       kind="ExternalOutput").ap()

    with tile.TileContext(nc) as tc:
        from contextlib import ExitStack
        with ExitStack() as ctx:
            const = ctx.enter_context(tc.tile_pool(name="const", bufs=1))
            xp = ctx.enter_context(tc.tile_pool(name="xp", bufs=4))
            pp = ctx.enter_context(tc.tile_pool(name="pp", bufs=2))
            vtp = ctx.enter_context(tc.tile_pool(name="vt", bufs=2))
            ep = ctx.enter_context(tc.tile_pool(name="ep", bufs=3))
            onp = ctx.enter_context(tc.tile_pool(name="on", bufs=2))
            rcp = ctx.enter_context(tc.tile_pool(name="rc", bufs=2))
            yp = ctx.enter_context(tc.tile_pool(name="yp", bufs=2))
            psS = ctx.enter_context(
                tc.tile_pool(name="psS", bufs=2, space="PSUM"))
            psV = ctx.enter_context(
                tc.tile_pool(name="psV", bufs=2, space="PSUM"))
            psAcc = ctx.enter_context(
                tc.tile_pool(name="psAcc", bufs=1, space="PSUM"))

            # ---- first unit's x (split across DMA queues), then weights ----
            x2 = []
            for u in range(N_UNITS):
                t = xp.tile([128, NT], F16, tag="x2", name=f"x2_{u}")
                x2.append(t)
            for q in range(4):
                r = slice(32 * q, 32 * (q + 1))
                nc.sync.dma_start(x2[0][r, :], xs[0][r, :])
            wA_sb, wv_sb, wo_sb = [], [], []
            for p in range(2):
                t = const.tile([128, GD], F16, tag=f"wA{p}", name=f"wA{p}")
                nc.sync.dma_start(t[:], wAT[p])
                wA_sb.append(t)
                t = const.tile([128, GD], F16, tag=f"wv{p}", name=f"wv{p}")
                nc.sync.dma_start(t[:], wvT[p])
                wv_sb.append(t)
            for k in range(2):
                t = const.tile([128, 256], F16, tag=f"wo{k}", name=f"wo{k}")
                nc.sync.dma_start(t[:], woT[k])
                wo_sb.append(t)
            # selector for denominator broadcast: psR = sel.T @ den2
            # maps den row 0 -> out rows 0..63, den row 32 -> rows 64..127
            sel = const.tile([33, 128], F16, tag="sel", name="sel")
            nc.gpsimd.memset(sel[:], 0.0)
            nc.vector.memset(sel[0:1, 0:64], 1.0)
            nc.vector.memset(sel[32:33, 64:128], 1.0)
            bias_m2 = const.tile([128, 1], F32, tag="biasm2", name="bias_m2")
            nc.vector.memset(bias_m2[:], -2.0)
            # zero the den ring buffers once: rows 1..31 stay 0 forever, so
            # the 0*garbage -> NaN path through the sel matmul can't occur
            for nh in range(2):
                for i in range(2):
                    d0 = rcp.tile([33, 512], F32, tag=f"den{nh}",
                                  name=f"dinit{nh}_{i}")
                    nc.gpsimd.memset(d0[:], 1.0)

            # ---- prefetch remaining x, split across DMA queues ----
            for u in range(1, N_UNITS):
                for q in range(4):
                    r = slice(32 * q, 32 * (q + 1))
                    nc.sync.dma_start(x2[u][r, :], xs[u][r, :])

            # per-unit state (filled by proj/v emitters)
            P2h = [None] * N_UNITS
            vts = [None] * N_UNITS
            outN = [None] * N_UNITS

            def proj_mms(u, nh):
                # P = A^T.T @ x : two 64-row bands on the PE concurrently
                p = u % 2
                pv = psV.tile([128, 512], F32, tag="pvv",
                              name=f"pj_{u}_{nh}")
                s = ts(nh, 512)
                nc.tensor.matmul(
                    pv[0:64, :], wA_sb[p][0:64, :], x2[u][0:64, s],
                    start=True, stop=True, tile_position=(0, 0))
                nc.tensor.matmul(
                    pv[64:128, :], wA_sb[p][64:128, :], x2[u][64:128, s],
                    start=True, stop=True, tile_position=(64, 64))
                return pv

            def proj_copy(u, nh, pv):
                if P2h[u] is None:
                    P2h[u] = pp.tile([128, NT], F16, tag="P2", name=f"P2_{u}")
                nc.vector.tensor_copy(P2h[u][:, ts(nh, 512)], pv[:])

            def v_alloc(u, g, state):
                if vts[u] is None:
                    vv = [vtp.tile([128, 8, GD + 1], F16, tag=f"vt{gg}",
                                   name=f"vt{gg}_{u}") for gg in range(2)]
                    for gg in range(2):
                        nc.vector.memset(vv[gg][:, :, GD:GD + 1], 1.0)
                    vts[u] = vv
                state[f"pvg{g}"] = psV.tile([128, 8, GD], F32, tag="pvv",
                                            name=f"pvv{g}_{u}")

            def v_mms(u, g, pvg, mc0, mc1):
                p = u % 2
                for mc in range(mc0, mc1):
                    nc.tensor.matmul(
                        pvg[:, mc, :],
                        x2[u][64 * g:64 * (g + 1), ts(mc, 128)],
                        wv_sb[p][64 * g:64 * (g + 1), :],
                        start=True, stop=True,
                        tile_position=(64 * g, 0))

            def v_cast(u, g, pvg):
                nc.vector.tensor_copy(vts[u][g][:, :, 0:GD], pvg[:])

            def make_unit_ops(u):
                """List of callables emitting unit u's prep, for injection."""
                state = {}

                def mk_proj(nh):
                    def f():
                        state[f"pv{nh}"] = proj_mms(u, nh)
                    return f

                def mk_pcopy(nh):
                    return lambda: proj_copy(u, nh, state[f"pv{nh}"])

                def mk_valloc(g):
                    return lambda: v_alloc(u, g, state)

                def mk_vmm(g, a, b):
                    return lambda: v_mms(u, g, state[f"pvg{g}"], a, b)

                def mk_vcast(g):
                    return lambda: v_cast(u, g, state[f"pvg{g}"])

                return [mk_proj(0), mk_pcopy(0), mk_proj(1), mk_pcopy(1),
                        mk_valloc(0), mk_vmm(0, 0, 4), mk_vmm(0, 4, 8),
                        mk_vcast(0),
                        mk_valloc(1), mk_vmm(1, 0, 4), mk_vmm(1, 4, 8),
                        mk_vcast(1)]

            def attention(u, nh, inject):
                ns = ts(nh, 512)
                psO = [psAcc.tile([GD + 1, 512], F32, tag=f"psO{g}",
                                  name=f"psO{g}_{u}_{nh}") for g in range(2)]
                sc = {}
                for step in range(10):
                    if step < 8:
                        msl = ts(step, 128)
                        ps = psS.tile([128, NT], F32, tag="pss",
                                      name=f"pss_{u}_{nh}_{step}")
                        nc.tensor.matmul(
                            ps[:, 0:512], x2[u][0:64, msl], P2h[u][0:64, ns],
                            start=True, stop=True, tile_position=(0, 0))
                        nc.tensor.matmul(
                            ps[:, 512:1024], x2[u][64:128, msl],
                            P2h[u][64:128, ns],
                            start=True, stop=True, tile_position=(64, 0))
                        sc[step] = ps
                    if step >= 2:
                        mc = step - 2
                        E2 = ep.tile([128, NT], F16, tag="E2",
                                     name=f"E2_{u}_{nh}_{mc}")
                        # bias -2 keeps exp(score) well inside f16 range
                        # (max |score| ~10.5); cancels in the softmax ratio
                        nc.scalar.activation(E2[:], sc.pop(mc)[:], EXP,
                                             scale=0.125, bias=bias_m2[:])
                        nc.tensor.matmul(
                            psO[0][:], vts[u][0][:, mc, :], E2[:, 0:512],
                            start=(mc == 0), stop=(mc == 7))
                        nc.tensor.matmul(
                            psO[1][:], vts[u][1][:, mc, :], E2[:, 512:1024],
                            start=(mc == 0), stop=(mc == 7))
                    if inject:
                        inject.pop(0)()
                while inject:
                    inject.pop(0)()
                # tails: stage numerators, reciprocal of denominators,
                # broadcast+normalize on gpsimd (off the PE path)
                den2 = rcp.tile([33, 512], F32, tag=f"den{nh}",
                                name=f"den{nh}_{u}")
                # reciprocal chain first (it gates the sel matmul + final
                # multiply); numerator staging can run behind it.  The
                # custom-DVE reciprocal needs raw fp32 bits, so the PSUM
                # denominator rows are staged into SBUF.
                for g in range(2):
                    nc.vector.tensor_copy(
                        den2[32 * g:32 * g + 1, :], psO[g][GD:GD + 1, :])
                rec2 = rcp.tile([33, 512], F32, tag=f"rec{nh}",
                                name=f"rec{nh}_{u}")
                nc.vector.reciprocal_approx_fast(rec2[:], den2[:])
                rec2h = rcp.tile([33, 512], F16, tag=f"rech{nh}",
                                 name=f"rech{nh}_{u}")
                nc.vector.tensor_copy(rec2h[:], rec2[:])
                for g in range(2):
                    # last unit: ACT is idle after the final exp, DVE is the
                    # tail critical path -> stage numerators on ACT there
                    eng = nc.scalar if u == N_UNITS - 1 else nc.vector
                    if eng is nc.scalar:
                        eng.copy(outN[u][GD * g:GD * (g + 1), ns],
                                 psO[g][0:GD, :])
                    else:
                        eng.tensor_copy(outN[u][GD * g:GD * (g + 1), ns],
                                        psO[g][0:GD, :])
                # broadcast recips to all 128 rows via tiny K=33 matmul,
                # then one DVE multiply normalizes both groups
                psR = psV.tile([128, 512], F32, tag="pvv",
                               name=f"psR{nh}_{u}")
                nc.tensor.matmul(psR[:], sel[:], rec2h[:],
                                 start=True, stop=True)
                rows = outN[u][:, ns]
                nc.vector.tensor_mul(rows, rows, psR[:])

            def out_proj_block(b, ec):
                yt = yp.tile([128, NT], F16, tag="yt", name=f"yt_{b}_{ec}")
                for nh in range(2):
                    s = ts(nh, 512)
                    psY = psV.tile([128, 512], F32, tag="pvv",
                                   name=f"psY_{b}_{ec}_{nh}")
                    for kc in range(2):
                        nc.tensor.matmul(
                            psY[:], wo_sb[kc][:, ts(ec, 128)],
                            outN[2 * b + kc][:, s],
                            start=(kc == 0), stop=(kc == 1))
                    nc.vector.tensor_copy(yt[:, s], psY[:])
                for q in range(8):
                    r = slice(16 * q, 16 * (q + 1))
                    nc.sync.dma_start(y[b][16 * q + 128 * ec:
                                           16 * (q + 1) + 128 * ec, :],
                                      yt[r, :])

            # ---- schedule ----
            for op in make_unit_ops(0):
                op()
            for u in range(N_UNITS):
                outN[u] = onp.tile([128, NT], F16, tag=f"on{u % 2}",
                                   name=f"outN_{u}")
                inj = make_unit_ops(u + 1) if u + 1 < N_UNITS else []
                attention(u, 0, inj)
                if u == 2:
                    out_proj_block(0, 0)
                attention(u, 1, [])
                if u == 2:
                    out_proj_block(0, 1)
            out_proj_block(1, 0)
            out_proj_block(1, 1)

    nc.finalize()
    return nc


_NC_CACHE = None


def _get_nc():
    global _NC_CACHE
    if _NC_CACHE is None:
        _NC_CACHE = _build_program()
    return _NC_CACHE


def _prep_inputs(x, wq, wk, wv, wo):
    B = x.shape[0]
    xr = np.ascontiguousarray(
        x.reshape(B // B_PER_CORE, N_UNITS, 128, NT), dtype=np.float16)
    # A^T = wq^T wk per group, stacked per pair: [2, 128, GD]
    wAT_np = np.einsum('gdc,gde->gce', wq, wk)  # [G, c, e]
    wAT_np = np.ascontiguousarray(
        wAT_np.reshape(2, 128, GD), dtype=np.float16)
    wvT_np = np.ascontiguousarray(
        wv.transpose(0, 2, 1).reshape(2, 128, GD), dtype=np.float16)
    woT_np = np.ascontiguousarray(wo.T.reshape(2, 128, 256), dtype=np.float16)
    return xr, wAT_np, wvT_np, woT_np


def run(x, wq, wk, wv, wo, trace=False, **trace_kwargs):
    x = np.asarray(x, dtype=np.float32)
    B, C, H, W = x.shape
    xr, wAT_np, wvT_np, woT_np = _prep_inputs(
        x, np.asarray(wq, np.float32), np.asarray(wk, np.float32),
        np.asarray(wv, np.float32), np.asarray(wo, np.float32))
    in_maps = []
    for c in range(N_CORES):
        in_maps.append({
            "xs": xr[c],
            "wAT": wAT_np, "wvT": wvT_np, "woT": woT_np,
        })
    res = run_bass_kernel_spmd(_get_nc(), in_maps, list(range(N_CORES)),
                               trace=trace, **trace_kwargs)
    outs = [res.results[c]["y"] for c in range(N_CORES)]
    yfull = np.concatenate(outs, axis=0).reshape(B, C, H, W)
    return yfull.astype(np.float32), res


def kernel(x, wq, wk, wv, wo):
    out, _ = run(x, wq, wk, wv, wo, trace=False)
    return out


# revision 33
# speedup vs baseline: 1.0303x; 1.0303x over previous
"""GroupMixAttention Trainium2 kernel (8-core SPMD, batch-parallel), v2.

Problem: x[16,256,32,32]; per group g (4 groups of 64 ch):
  Q/K/V = wq/wk/wv[g] @ xg   (xg = [64, 1024])
  scores = (Q^T K)/8 ; attn = softmax(scores, -1) ; out = V @ attn^T
then y = wo @ concat(out).

Sharding: data-parallel over batch, 2 batches per core, no collectives.

v2 design (from baseline trace analysis: PE busy 170us of 211us span,
everything fp32 at half stream rate; ACT exp 79us):
  - whole pipeline in fp16 (host casts inputs): matmuls stream at full
    rate (1 col/cycle @2.4GHz vs 1/2 for fp32).
  - K projection eliminated algebraically: scores^T = K^T Q
    = x^T (wk^T wq) x, so host precomputes A^T = wq^T wk per group and
    the kernel computes P = A x (one projection) then scores chunks
    directly from raw x chunks as lhsT.
  - both groups of a pair are packed in PE row-bands (tile_position
    (0,0)/(64,0)) writing the two 512-col halves of one [128,1024]
    PSUM tile -> ONE exp per m-chunk step (halves ACT instr overhead).
  - attention inner loop is ACT(exp)-bound (~1.05us/step vs ~0.85us of
    PE work); the next unit's P-projection and V^T matmuls are
    software-injected into the loop's PE slack, and out_proj of the
    previous batch is placed at nh boundaries where the exp pipeline
    drains anyway.
  - softmax denominators via ones-column in the PV lhsT (row 64 of
    psO); normalization = DVE reciprocal_approx_fast on the den row +
    gpsimd partition_broadcast + gpsimd multiply (gpsimd is otherwise
    idle; PE no longer does broadcast matmuls).
  - V^T chunks accumulate into [128,8,64] PSUM tiles; one strided DVE
    cast per group fills the [128,8,65] fp16 lhsT (ones col memset).
"""

import os
import sys

import numpy as np

for _p in ("/opt/trn_rl_repo", "/root/.axon_site/_ro/trn_rl_repo"):
    if os.path.isdir(_p) and _p not in sys.path:
        sys.path.insert(0, _p)

import concourse.bass as bass
import concourse.mybir as mybir
import concourse.tile as tile
from concourse import bacc
from concourse.bass_utils import run_bass_kernel_spmd

F32 = mybir.dt.float32
F16 = mybir.dt.float16
EXP = mybir.ActivationFunctionType.Exp
N_CORES = 8
B_PER_CORE = 2  # 16 batches / 8 cores
N_UNITS = 4     # (batch, group-pair) units per core
NT = 1024       # H*W
GD = 64         # group dim
ts = bass.ts


def _build_program():
    nc = bacc.Bacc("TRN2", target_bir_lowering=False, debug=False,
                   num_devices=N_CORES)
    xs = nc.dram_tensor("xs", [N_UNITS, 128, NT], F16,
                        kind="ExternalInput").ap()
    wAT = nc.dram_tensor("wAT", [2, 128, GD], F16, kind="ExternalInput").ap()
    wvT = nc.dram_tensor("wvT", [2, 128, GD], F16, kind="ExternalInput").ap()
    woT = nc.dram_tensor("woT", [2, 128, 256], F16, kind="ExternalInput").ap()
    y = nc.dram_tensor("y", [B_PER_CORE, 256, NT], F16,
                       kind="ExternalOutput").ap()

    with tile.TileContext(nc) as tc:
        from contextlib import ExitStack
        with ExitStack() as ctx:
            const = ctx.enter_context(tc.tile_pool(name="const", bufs=1))
            xp = ctx.enter_context(tc.tile_pool(name="xp", bufs=4))
            pp = ctx.enter_context(tc.tile_pool(name="pp", bufs=2))
            vtp = ctx.enter_context(tc.tile_pool(name="vt", bufs=2))
            ep = ctx.enter_context(tc.tile_pool(name="ep", bufs=3))
            onp = ctx.enter_context(tc.tile_pool(name="on", bufs=2))
            rcp = ctx.enter_context(tc.tile_pool(name="rc", bufs=2))
            yp = ctx.enter_context(tc.tile_pool(name="yp", bufs=2))
            psS = ctx.enter_context(
                tc.tile_pool(name="psS", bufs=2, space="PSUM"))
            psV = ctx.enter_context(
                tc.tile_pool(name="psV", bufs=2, space="PSUM"))
            psAcc = ctx.enter_context(
                tc.tile_pool(name="psAcc", bufs=1, space="PSUM"))

            # ---- first unit's x (split across DMA queues), then weights ----
            x2 = []
            for u in range(N_UNITS):
                t = xp.tile([128, NT], F16, tag="x2", name=f"x2_{u}")
                x2.append(t)
            for q in range(4):
                r = slice(32 * q, 32 * (q + 1))
                nc.sync.dma_start(x2[0][r, :], xs[0][r, :])
            wA_sb, wv_sb, wo_sb = [], [], []
            for p in range(2):
                t = const.tile([128, GD], F16, tag=f"wA{p}", name=f"wA{p}")
                nc.sync.dma_start(t[:], wAT[p])
                wA_sb.append(t)
                t = const.tile([128, GD], F16, tag=f"wv{p}", name=f"wv{p}")
                nc.sync.dma_start(t[:], wvT[p])
                wv_sb.append(t)
            for k in range(2):
                t = const.tile([128, 256], F16, tag=f"wo{k}", name=f"wo{k}")
                nc.sync.dma_start(t[:], woT[k])
                wo_sb.append(t)
            # selector for denominator broadcast: psR = sel.T @ den2
            # maps den row 0 -> out rows 0..63, den row 32 -> rows 64..127
            sel = const.tile([33, 128], F16, tag="sel", name="sel")
            nc.gpsimd.memset(sel[:], 0.0)
            nc.vector.memset(sel[0:1, 0:64], 1.0)
            nc.vector.memset(sel[32:33, 64:128], 1.0)
            bias_m2 = const.tile([128, 1], F32, tag="biasm2", name="bias_m2")
            nc.vector.memset(bias_m2[:], -2.0)
            # zero the den ring buffers once: rows 1..31 stay 0 forever, so
            # the 0*garbage -> NaN path through the sel matmul can't occur
            for nh in range(2):
                for i in range(2):
                    d0 = rcp.tile([33, 512], F32, tag=f"den{nh}",
                                  name=f"dinit{nh}_{i}")
                    nc.gpsimd.memset(d0[:], 1.0)

            # ---- prefetch remaining x, split across DMA queues ----
            for u in range(1, N_UNITS):
                for q in range(4):
                    r = slice(32 * q, 32 * (q + 1))
                    nc.sync.dma_start(x2[u][r, :], xs[u][r, :])

            # per-unit state (filled by proj/v emitters)
            P2h = [None] * N_UNITS
            vts = [None] * N_UNITS
            outN = [None] * N_UNITS

            def proj_mms(u, nh):
                # P = A^T.T @ x : two 64-row bands on the PE concurrently
                p = u % 2
                pv = psV.tile([128, 512], F32, tag="pvv",
                              name=f"pj_{u}_{nh}")
                s = ts(nh, 512)
                nc.tensor.matmul(
                    pv[0:64, :], wA_sb[p][0:64, :], x2[u][0:64, s],
                    start=True, stop=True, tile_position=(0, 0))
                nc.tensor.matmul(
                    pv[64:128, :], wA_sb[p][64:128, :], x2[u][64:128, s],
                    start=True, stop=True, tile_position=(64, 64))
                return pv

            def proj_copy(u, nh, pv):
                if P2h[u] is None:
                    P2h[u] = pp.tile([128, NT], F16, tag="P2", name=f"P2_{u}")
                nc.vector.tensor_copy(P2h[u][:, ts(nh, 512)], pv[:])

            def v_alloc(u, g, state):
                if vts[u] is None:
                    vv = [vtp.tile([128, 8, GD + 1], F16, tag=f"vt{gg}",
                                   name=f"vt{gg}_{u}") for gg in range(2)]
                    for gg in range(2):
                        nc.vector.memset(vv[gg][:, :, GD:GD + 1], 1.0)
                    vts[u] = vv
                state[f"pvg{g}"] = psV.tile([128, 8, GD], F32, tag="pvv",
                                            name=f"pvv{g}_{u}")

            def v_mms(u, g, pvg, mc0, mc1):
                p = u % 2
                for mc in range(mc0, mc1):
                    nc.tensor.matmul(
                        pvg[:, mc, :],
                        x2[u][64 * g:64 * (g + 1), ts(mc, 128)],
                        wv_sb[p][64 * g:64 * (g + 1), :],
                        start=True, stop=True,
                        tile_position=(64 * g, 0))

            def v_cast(u, g, pvg):
                nc.vector.tensor_copy(vts[u][g][:, :, 0:GD], pvg[:])

            def make_unit_ops(u):
                """List of callables emitting unit u's prep, for injection."""
                state = {}

                def mk_proj(nh):
                    def f():
                        state[f"pv{nh}"] = proj_mms(u, nh)
                    return f

                def mk_pcopy(nh):
                    return lambda: proj_copy(u, nh, state[f"pv{nh}"])

                def mk_valloc(g):
                    return lambda: v_alloc(u, g, state)

                def mk_vmm(g, a, b):
                    return lambda: v_mms(u, g, state[f"pvg{g}"], a, b)

                def mk_vcast(g):
                    return lambda: v_cast(u, g, state[f"pvg{g}"])

                return [mk_proj(0), mk_pcopy(0), mk_proj(1), mk_pcopy(1),
                        mk_valloc(0), mk_vmm(0, 0, 4), mk_vmm(0, 4, 8),
                        mk_vcast(0),
                        mk_valloc(1), mk_vmm(1, 0, 4), mk_vmm(1, 4, 8),
                        mk_vcast(1)]

            def attention(u, nh, inject):
                ns = ts(nh, 512)
                psO = [psAcc.tile([GD + 1, 512], F32, tag=f"psO{g}",
                                  name=f"psO{g}_{u}_{nh}") for g in range(2)]
                sc = {}
                for step in range(10):
                    if step < 8:
                        msl = ts(step, 128)
                        ps = psS.tile([128, NT], F32, tag="pss",
                                      name=f"pss_{u}_{nh}_{step}")
                        nc.tensor.matmul(
                            ps[:, 0:512], x2[u][0:64, msl], P2h[u][0:64, ns],
                            start=True, stop=True, tile_position=(0, 0))
                        nc.tensor.matmul(
                            ps[:, 512:1024], x2[u][64:128, msl],
                            P2h[u][64:128, ns],
                            start=True, stop=True, tile_position=(64, 0))
                        sc[step] = ps
                    if step >= 2:
                        mc = step - 2
                        E2 = ep.tile([128, NT], F16, tag="E2",
                                     name=f"E2_{u}_{nh}_{mc}")
                        # bias -2 keeps exp(score) well inside f16 range
                        # (max |score| ~10.5); cancels in the softmax ratio
                        nc.scalar.activation(E2[:], sc.pop(mc)[:], EXP,
                                             scale=0.125, bias=bias_m2[:])
                        nc.tensor.matmul(
                            psO[0][:], vts[u][0][:, mc, :], E2[:, 0:512],
                            start=(mc == 0), stop=(mc == 7))
                        nc.tensor.matmul(
                            psO[1][:], vts[u][1][:, mc, :], E2[:, 512:1024],
                            start=(mc == 0), stop=(mc == 7))
                    if inject:
                        inject.pop(0)()
                while inject:
                    inject.pop(0)()
                # tails: stage numerators, reciprocal of denominators,
                # broadcast+normalize on gpsimd (off the PE path)
                den2 = rcp.tile([33, 512], F32, tag=f"den{nh}",
                                name=f"den{nh}_{u}")
                for g in range(2):
                    nc.vector.tensor_copy(
                        outN[u][GD * g:GD * (g + 1), ns], psO[g][0:GD, :])
                    # custom-DVE reciprocal needs raw fp32 bits: stage the
                    # PSUM denominator row into SBUF first
                    nc.vector.tensor_copy(
                        den2[32 * g:32 * g + 1, :], psO[g][GD:GD + 1, :])
                rec2 = rcp.tile([33, 512], F32, tag=f"rec{nh}",
                                name=f"rec{nh}_{u}")
                nc.vector.reciprocal_approx_fast(rec2[:], den2[:])
                rec2h = rcp.tile([33, 512], F16, tag=f"rech{nh}",
                                 name=f"rech{nh}_{u}")
                nc.vector.tensor_copy(rec2h[:], rec2[:])
                # broadcast recips to all 128 rows via tiny K=33 matmul,
                # then one DVE multiply normalizes both groups
                psR = psV.tile([128, 512], F32, tag="pvv",
                               name=f"psR{nh}_{u}")
                nc.tensor.matmul(psR[:], sel[:], rec2h[:],
                                 start=True, stop=True)
                rows = outN[u][:, ns]
                nc.vector.tensor_mul(rows, rows, psR[:])

            def out_proj_block(b, ec):
                yt = yp.tile([128, NT], F16, tag="yt", name=f"yt_{b}_{ec}")
                for nh in range(2):
                    s = ts(nh, 512)
                    psY = psV.tile([128, 512], F32, tag="pvv",
                                   name=f"psY_{b}_{ec}_{nh}")
                    for kc in range(2):
                        nc.tensor.matmul(
                            psY[:], wo_sb[kc][:, ts(ec, 128)],
                            outN[2 * b + kc][:, s],
                            start=(kc == 0), stop=(kc == 1))
                    nc.vector.tensor_copy(yt[:, s], psY[:])
                for q in range(8):
                    r = slice(16 * q, 16 * (q + 1))
                    nc.sync.dma_start(y[b][16 * q + 128 * ec:
                                           16 * (q + 1) + 128 * ec, :],
                                      yt[r, :])

            # ---- schedule ----
            for op in make_unit_ops(0):
                op()
            for u in range(N_UNITS):
                outN[u] = onp.tile([128, NT], F16, tag=f"on{u % 2}",
                                   name=f"outN_{u}")
                inj = make_unit_ops(u + 1) if u + 1 < N_UNITS else []
                attention(u, 0, inj)
                if u == 2:
                    out_proj_block(0, 0)
                attention(u, 1, [])
                if u == 2:
                    out_proj_block(0, 1)
            out_proj_block(1, 0)
            out_proj_block(1, 1)

    nc.finalize()
    return nc


_NC_CACHE = None


def _get_nc():
    global _NC_CACHE
    if _NC_CACHE is None:
        _NC_CACHE = _build_program()
    return _NC_CACHE


def _prep_inputs(x, wq, wk, wv, wo):
    B = x.shape[0]
    xr = np.ascontiguousarray(
        x.reshape(B // B_PER_CORE, N_UNITS, 128, NT), dtype=np.float16)
    # A^T = wq^T wk per group, stacked per pair: [2, 128, GD]
    wAT_np = np.einsum('gdc,gde->gce', wq, wk)  # [G, c, e]
    wAT_np = np.ascontiguousarray(
        wAT_np.reshape(2, 128, GD), dtype=np.float16)
    wvT_np = np.ascontiguousarray(
        wv.transpose(0, 2, 1).reshape(2, 128, GD), dtype=np.float16)
    woT_np = np.ascontiguousarray(wo.T.reshape(2, 128, 256), dtype=np.float16)
    return xr, wAT_np, wvT_np, woT_np


def run(x, wq, wk, wv, wo, trace=False, **trace_kwargs):
    x = np.asarray(x, dtype=np.float32)
    B, C, H, W = x.shape
    xr, wAT_np, wvT_np, woT_np = _prep_inputs(
        x, np.asarray(wq, np.float32), np.asarray(wk, np.float32),
        np.asarray(wv, np.float32), np.asarray(wo, np.float32))
    in_maps = []
    for c in range(N_CORES):
        in_maps.append({
            "xs": xr[c],
            "wAT": wAT_np, "wvT": wvT_np, "woT": woT_np,
        })
    res = run_bass_kernel_spmd(_get_nc(), in_maps, list(range(N_CORES)),
                               trace=trace, **trace_kwargs)
    outs = [res.results[c]["y"] for c in range(N_CORES)]
    yfull = np.concatenate(outs, axis=0).reshape(B, C, H, W)
    return yfull.astype(np.float32), res


def kernel(x, wq, wk, wv, wo):
    out, _ = run(x, wq, wk, wv, wo, trace=False)
    return out


# revision 34
# speedup vs baseline: 1.0869x; 1.0550x over previous
"""GroupMixAttention Trainium2 kernel (8-core SPMD, batch-parallel), v2.

Problem: x[16,256,32,32]; per group g (4 groups of 64 ch):
  Q/K/V = wq/wk/wv[g] @ xg   (xg = [64, 1024])
  scores = (Q^T K)/8 ; attn = softmax(scores, -1) ; out = V @ attn^T
then y = wo @ concat(out).

Sharding: data-parallel over batch, 2 batches per core, no collectives.

v2 design (from baseline trace analysis: PE busy 170us of 211us span,
everything fp32 at half stream rate; ACT exp 79us):
  - whole pipeline in fp16 (host casts inputs): matmuls stream at full
    rate (1 col/cycle @2.4GHz vs 1/2 for fp32).
  - K projection eliminated algebraically: scores^T = K^T Q
    = x^T (wk^T wq) x, so host precomputes A^T = wq^T wk per group and
    the kernel computes P = A x (one projection) then scores chunks
    directly from raw x chunks as lhsT.
  - both groups of a pair are packed in PE row-bands (tile_position
    (0,0)/(64,0)) writing the two 512-col halves of one [128,1024]
    PSUM tile -> ONE exp per m-chunk step (halves ACT instr overhead).
  - attention inner loop is ACT(exp)-bound (~1.05us/step vs ~0.85us of
    PE work); the next unit's P-projection and V^T matmuls are
    software-injected into the loop's PE slack, and out_proj of the
    previous batch is placed at nh boundaries where the exp pipeline
    drains anyway.
  - softmax denominators via ones-column in the PV lhsT (row 64 of
    psO); normalization = DVE reciprocal_approx_fast on the den row +
    gpsimd partition_broadcast + gpsimd multiply (gpsimd is otherwise
    idle; PE no longer does broadcast matmuls).
  - V^T chunks accumulate into [128,8,64] PSUM tiles; one strided DVE
    cast per group fills the [128,8,65] fp16 lhsT (ones col memset).
"""

import os
import sys

import numpy as np

for _p in ("/opt/trn_rl_repo", "/root/.axon_site/_ro/trn_rl_repo"):
    if os.path.isdir(_p) and _p not in sys.path:
        sys.path.insert(0, _p)

import concourse.bass as bass
import concourse.mybir as mybir
import concourse.tile as tile
from concourse import bacc
from concourse.bass_utils import run_bass_kernel_spmd

F32 = mybir.dt.float32
F16 = mybir.dt.float16
EXP = mybir.ActivationFunctionType.Exp
N_CORES = 8
B_PER_CORE = 2  # 16 batches / 8 cores
N_UNITS = 4     # (batch, group-pair) units per core
NT = 1024       # H*W
GD = 64         # group dim
ts = bass.ts


def _build_program():
    nc = bacc.Bacc("TRN2", target_bir_lowering=False, debug=False,
                   num_devices=N_CORES)
    xs = nc.dram_tensor("xs", [N_UNITS, 128, NT], F16,
                        kind="ExternalInput").ap()
    wAT = nc.dram_tensor("wAT", [2, 128, GD], F16, kind="ExternalInput").ap()
    wvT = nc.dram_tensor("wvT", [2, 128, GD], F16, kind="ExternalInput").ap()
    woT = nc.dram_tensor("woT", [2, 128, 256], F16, kind="ExternalInput").ap()
    y = nc.dram_tensor("y", [B_PER_CORE, 256, NT], F16,
                       kind="ExternalOutput").ap()

    with tile.TileContext(nc) as tc:
        from contextlib import ExitStack
        with ExitStack() as ctx:
            const = ctx.enter_context(tc.tile_pool(name="const", bufs=1))
            xp = ctx.enter_context(tc.tile_pool(name="xp", bufs=4))
            pp = ctx.enter_context(tc.tile_pool(name="pp", bufs=2))
            vtp = ctx.enter_context(tc.tile_pool(name="vt", bufs=2))
            ep = ctx.enter_context(tc.tile_pool(name="ep", bufs=3))
            onp = ctx.enter_context(tc.tile_pool(name="on", bufs=2))
            rcp = ctx.enter_context(tc.tile_pool(name="rc", bufs=2))
            yp = ctx.enter_context(tc.tile_pool(name="yp", bufs=2))
            psS = ctx.enter_context(
                tc.tile_pool(name="psS", bufs=2, space="PSUM"))
            psV = ctx.enter_context(
                tc.tile_pool(name="psV", bufs=2, space="PSUM"))
            psAcc = ctx.enter_context(
                tc.tile_pool(name="psAcc", bufs=1, space="PSUM"))

            # ---- first unit's x (split across DMA queues), then weights ----
            x2 = []
            for u in range(N_UNITS):
                t = xp.tile([128, NT], F16, tag="x2", name=f"x2_{u}")
                x2.append(t)
            for q in range(4):
                r = slice(32 * q, 32 * (q + 1))
                nc.sync.dma_start(x2[0][r, :], xs[0][r, :])
            wA_sb, wv_sb, wo_sb = [], [], []
            for p in range(2):
                t = const.tile([128, GD], F16, tag=f"wA{p}", name=f"wA{p}")
                nc.sync.dma_start(t[:], wAT[p])
                wA_sb.append(t)
                t = const.tile([128, GD], F16, tag=f"wv{p}", name=f"wv{p}")
                nc.sync.dma_start(t[:], wvT[p])
                wv_sb.append(t)
            for k in range(2):
                t = const.tile([128, 256], F16, tag=f"wo{k}", name=f"wo{k}")
                nc.sync.dma_start(t[:], woT[k])
                wo_sb.append(t)
            # selector for denominator broadcast: psR = sel.T @ den2
            # maps den row 0 -> out rows 0..63, den row 32 -> rows 64..127
            sel = const.tile([33, 128], F16, tag="sel", name="sel")
            nc.gpsimd.memset(sel[:], 0.0)
            nc.vector.memset(sel[0:1, 0:64], 1.0)
            nc.vector.memset(sel[32:33, 64:128], 1.0)
            bias_m2 = const.tile([128, 1], F32, tag="biasm2", name="bias_m2")
            nc.vector.memset(bias_m2[:], -2.0)
            # zero the den ring buffers once: rows 1..31 stay 0 forever, so
            # the 0*garbage -> NaN path through the sel matmul can't occur
            for nh in range(2):
                for i in range(2):
                    d0 = rcp.tile([33, 512], F32, tag=f"den{nh}",
                                  name=f"dinit{nh}_{i}")
                    nc.gpsimd.memset(d0[:], 1.0)

            # ---- prefetch remaining x, split across DMA queues ----
            for u in range(1, N_UNITS):
                for q in range(4):
                    r = slice(32 * q, 32 * (q + 1))
                    nc.sync.dma_start(x2[u][r, :], xs[u][r, :])

            # per-unit state (filled by proj/v emitters)
            P2h = [None] * N_UNITS
            vts = [None] * N_UNITS
            outN = [None] * N_UNITS

            def proj_mms(u, nh):
                # P = A^T.T @ x : two 64-row bands on the PE concurrently
                p = u % 2
                pv = psV.tile([128, 512], F32, tag="pvv",
                              name=f"pj_{u}_{nh}")
                s = ts(nh, 512)
                nc.tensor.matmul(
                    pv[0:64, :], wA_sb[p][0:64, :], x2[u][0:64, s],
                    start=True, stop=True, tile_position=(0, 0))
                nc.tensor.matmul(
                    pv[64:128, :], wA_sb[p][64:128, :], x2[u][64:128, s],
                    start=True, stop=True, tile_position=(64, 64))
                return pv

            def proj_copy(u, nh, pv):
                if P2h[u] is None:
                    P2h[u] = pp.tile([128, NT], F16, tag="P2", name=f"P2_{u}")
                nc.vector.tensor_copy(P2h[u][:, ts(nh, 512)], pv[:])

            def v_alloc(u, g, state):
                if vts[u] is None:
                    vv = [vtp.tile([128, 8, GD + 1], F16, tag=f"vt{gg}",
                                   name=f"vt{gg}_{u}") for gg in range(2)]
                    for gg in range(2):
                        nc.vector.memset(vv[gg][:, :, GD:GD + 1], 1.0)
                    vts[u] = vv
                state[f"pvg{g}"] = psV.tile([128, 8, GD], F32, tag="pvv",
                                            name=f"pvv{g}_{u}")

            def v_mms(u, g, pvg, mc0, mc1):
                p = u % 2
                for mc in range(mc0, mc1):
                    nc.tensor.matmul(
                        pvg[:, mc, :],
                        x2[u][64 * g:64 * (g + 1), ts(mc, 128)],
                        wv_sb[p][64 * g:64 * (g + 1), :],
                        start=True, stop=True,
                        tile_position=(64 * g, 0))

            def v_cast(u, g, pvg):
                nc.vector.tensor_copy(vts[u][g][:, :, 0:GD], pvg[:])

            def make_unit_ops(u):
                """List of callables emitting unit u's prep, for injection."""
                state = {}

                def mk_proj(nh):
                    def f():
                        state[f"pv{nh}"] = proj_mms(u, nh)
                    return f

                def mk_pcopy(nh):
                    return lambda: proj_copy(u, nh, state[f"pv{nh}"])

                def mk_valloc(g):
                    return lambda: v_alloc(u, g, state)

                def mk_vmm(g, a, b):
                    return lambda: v_mms(u, g, state[f"pvg{g}"], a, b)

                def mk_vcast(g):
                    return lambda: v_cast(u, g, state[f"pvg{g}"])

                return [mk_proj(0), mk_pcopy(0), mk_proj(1), mk_pcopy(1),
                        mk_valloc(0), mk_vmm(0, 0, 4), mk_vmm(0, 4, 8),
                        mk_vcast(0),
                        mk_valloc(1), mk_vmm(1, 0, 4), mk_vmm(1, 4, 8),
                        mk_vcast(1)]

            def attention(u, nh, inject):
                ns = ts(nh, 512)
                psO = [psAcc.tile([GD + 1, 512], F32, tag=f"psO{g}",
                                  name=f"psO{g}_{u}_{nh}") for g in range(2)]
                sc = {}
                for step in range(10):
                    if step < 8:
                        msl = ts(step, 128)
                        ps = psS.tile([128, NT], F32, tag="pss",
                                      name=f"pss_{u}_{nh}_{step}")
                        nc.tensor.matmul(
                            ps[:, 0:512], x2[u][0:64, msl], P2h[u][0:64, ns],
                            start=True, stop=True, tile_position=(0, 0))
                        nc.tensor.matmul(
                            ps[:, 512:1024], x2[u][64:128, msl],
                            P2h[u][64:128, ns],
                            start=True, stop=True, tile_position=(64, 0))
                        sc[step] = ps
                    if step >= 2:
                        mc = step - 2
                        E2 = ep.tile([128, NT], F16, tag="E2",
                                     name=f"E2_{u}_{nh}_{mc}")
                        # bias -2 keeps exp(score) well inside f16 range
                        # (max |score| ~10.5); cancels in the softmax ratio
                        nc.scalar.activation(E2[:], sc.pop(mc)[:], EXP,
                                             scale=0.125, bias=bias_m2[:])
                        nc.tensor.matmul(
                            psO[0][:], vts[u][0][:, mc, :], E2[:, 0:512],
                            start=(mc == 0), stop=(mc == 7))
                        nc.tensor.matmul(
                            psO[1][:], vts[u][1][:, mc, :], E2[:, 512:1024],
                            start=(mc == 0), stop=(mc == 7))
                    if inject:
                        inject.pop(0)()
                while inject:
                    inject.pop(0)()
                # tails: stage numerators, reciprocal of denominators,
                # broadcast+normalize on gpsimd (off the PE path)
                den2 = rcp.tile([33, 512], F32, tag=f"den{nh}",
                                name=f"den{nh}_{u}")
                for g in range(2):
                    nc.vector.tensor_copy(
                        outN[u][GD * g:GD * (g + 1), ns], psO[g][0:GD, :])
                    # custom-DVE reciprocal needs raw fp32 bits: stage the
                    # PSUM denominator row into SBUF first
                    nc.vector.tensor_copy(
                        den2[32 * g:32 * g + 1, :], psO[g][GD:GD + 1, :])
                rec2 = rcp.tile([33, 512], F32, tag=f"rec{nh}",
                                name=f"rec{nh}_{u}")
                nc.vector.reciprocal_approx_fast(rec2[:], den2[:])
                rec2h = rcp.tile([33, 512], F16, tag=f"rech{nh}",
                                 name=f"rech{nh}_{u}")
                nc.vector.tensor_copy(rec2h[:], rec2[:])
                # broadcast recips to all 128 rows via tiny K=33 matmul,
                # then one DVE multiply normalizes both groups
                psR = psV.tile([128, 512], F32, tag="pvv",
                               name=f"psR{nh}_{u}")
                nc.tensor.matmul(psR[:], sel[:], rec2h[:],
                                 start=True, stop=True)
                rows = outN[u][:, ns]
                nc.vector.tensor_mul(rows, rows, psR[:])

            def out_proj_block(b, ec):
                yt = yp.tile([128, NT], F16, tag="yt", name=f"yt_{b}_{ec}")
                for nh in range(2):
                    s = ts(nh, 512)
                    psY = psV.tile([128, 512], F32, tag="pvv",
                                   name=f"psY_{b}_{ec}_{nh}")
                    for kc in range(2):
                        nc.tensor.matmul(
                            psY[:], wo_sb[kc][:, ts(ec, 128)],
                            outN[2 * b + kc][:, s],
                            start=(kc == 0), stop=(kc == 1))
                    nc.vector.tensor_copy(yt[:, s], psY[:])
                for q in range(4):
                    r = slice(32 * q, 32 * (q + 1))
                    nc.sync.dma_start(y[b][32 * q + 128 * ec:
                                           32 * (q + 1) + 128 * ec, :],
                                      yt[r, :])

            # ---- schedule ----
            for op in make_unit_ops(0):
                op()
            for u in range(N_UNITS):
                outN[u] = onp.tile([128, NT], F16, tag=f"on{u % 2}",
                                   name=f"outN_{u}")
                inj = make_unit_ops(u + 1) if u + 1 < N_UNITS else []
                attention(u, 0, inj)
                if u == 2:
                    out_proj_block(0, 0)
                attention(u, 1, [])
                if u == 2:
                    out_proj_block(0, 1)
            out_proj_block(1, 0)
            out_proj_block(1, 1)

    nc.finalize()
    return nc


_NC_CACHE = None


def _get_nc():
    global _NC_CACHE
    if _NC_CACHE is None:
        _NC_CACHE = _build_program()
    return _NC_CACHE


def _prep_inputs(x, wq, wk, wv, wo):
    B = x.shape[0]
    xr = np.ascontiguousarray(
        x.reshape(B // B_PER_CORE, N_UNITS, 128, NT), dtype=np.float16)
    # A^T = wq^T wk per group, stacked per pair: [2, 128, GD]
    wAT_np = np.einsum('gdc,gde->gce', wq, wk)  # [G, c, e]
    wAT_np = np.ascontiguousarray(
        wAT_np.reshape(2, 128, GD), dtype=np.float16)
    wvT_np = np.ascontiguousarray(
        wv.transpose(0, 2, 1).reshape(2, 128, GD), dtype=np.float16)
    woT_np = np.ascontiguousarray(wo.T.reshape(2, 128, 256), dtype=np.float16)
    return xr, wAT_np, wvT_np, woT_np


def run(x, wq, wk, wv, wo, trace=False, **trace_kwargs):
    x = np.asarray(x, dtype=np.float32)
    B, C, H, W = x.shape
    xr, wAT_np, wvT_np, woT_np = _prep_inputs(
        x, np.asarray(wq, np.float32), np.asarray(wk, np.float32),
        np.asarray(wv, np.float32), np.asarray(wo, np.float32))
    in_maps = []
    for c in range(N_CORES):
        in_maps.append({
            "xs": xr[c],
            "wAT": wAT_np, "wvT": wvT_np, "woT": woT_np,
        })
    res = run_bass_kernel_spmd(_get_nc(), in_maps, list(range(N_CORES)),
                               trace=trace, **trace_kwargs)
    outs = [res.results[c]["y"] for c in range(N_CORES)]
    yfull = np.concatenate(outs, axis=0).reshape(B, C, H, W)
    return yfull.astype(np.float32), res


def kernel(x, wq, wk, wv, wo):
    out, _ = run(x, wq, wk, wv, wo, trace=False)
    return out
